# revision 37
# baseline (speedup 1.0000x reference)
"""GAT (3-layer, 4-head) forward on 8 Trainium2 NeuronCores.

Strategy: nodes are partitioned across the 8 cores (destination-sharded);
each core processes the in-edges of its nodes in a padded block layout
[128 dst nodes (partitions) x W in-edge slots (free dim)], gathering source
node rows with one dma_gather per (block, table-half) from a replicated
node table in DRAM.  Table rows are 512B: 240 bf16 h channels + 16 fp8 h
channels + 4 f32 a_src scores.  Self-loops ride as slot 0 of each region;
padding slots point at a dedicated pad row (h=0, a_src=-3e4 so exp->0),
which removes all masking.  Per-layer node features are produced by a
sharded dense matmul and exchanged with an AllGather; softmax denominators
come free from the scalar engine's activation accumulator.

Self-contained: builds/compiles the Bass program on first call from the
actual inputs, runs SPMD on cores 0-7, reassembles the full output.
"""

import sys

for _p in ("/opt/trn_rl_repo",):
    if _p not in sys.path:
        sys.path.insert(0, _p)

import numpy as np

import concourse.bass as bass
import concourse.mybir as mybir
import concourse.tile as tile
from concourse import bacc, bass_utils

F32 = mybir.dt.float32
BF = mybir.dt.bfloat16
FP8 = mybir.dt.float8e4
I16 = mybir.dt.int16
AX = mybir.AluOpType
ACT = mybir.ActivationFunctionType

NC = 8          # cores
P = 128         # partitions / block size
H, C = 4, 64    # heads, channels
HC = H * C      # 256
EXT = HC + 2 * H          # 264 = h | a_src | a_dst (f32, dense-phase psum)
ROW = 256                 # table row in bf16 units (512B)
NBF = 240                 # h channels stored in bf16
NF8 = 16                  # h channels stored in fp8 (bf16 slots 240:248)
# bf16 slots 248:256 hold 4 f32 a_src values
NEG_SLOPE = 0.2
PAD_SCORE = -30000.0

# feature toggles (for HW bisection)
USE_SCALAR_LRELU = False   # scalar-engine Lrelu w/ alpha vs vector mult+max
USE_ACCUM = False          # Exp accum_out for softmax denom vs vector reduce
USE_SCALAR_COPIES = True  # PSUM->SBUF packing copies on scalar vs vector
GCAP = 8                   # max gather width (slots); >8-wide gathers hang HW

_cache = {}


def _build_wext(w, att_src, att_dst):
    # h = x@w ; a_src[n,h] = sum_c h[n,h*C+c]*att_src[h,c]  ->  x @ (w @ M)
    m_src = np.zeros((HC, H), np.float32)
    m_dst = np.zeros((HC, H), np.float32)
    for hh in range(H):
        m_src[hh * C:(hh + 1) * C, hh] = att_src[hh]
        m_dst[hh * C:(hh + 1) * C, hh] = att_dst[hh]
    return np.concatenate([w, w @ m_src, w @ m_dst], axis=1).astype(np.float32)  # [din, 264]


def _host_prep(x, edge_index, params):
    N = x.shape[0]
    IN = x.shape[1]
    src = np.asarray(edge_index[0], np.int64).astype(np.int32)
    dst = np.asarray(edge_index[1], np.int64).astype(np.int32)

    half_id = N // 2                       # node-id split for lo/hi tables
    KB = -(-(N - half_id) // (P * (NC // 2)))   # blocks per core (per half)
    CH_CAP = KB * P                        # rows per core in table
    TAB = NC * CH_CAP
    HALFT = (NC // 2) * CH_CAP             # table rows in lo half
    LO_PAD = HALFT - 1                     # local idx of pad row in lo table
    HI_PAD = TAB - HALFT - 1               # local idx of pad row in hi table

    lo_deg = np.bincount(dst[src < half_id], minlength=N)
    hi_deg = np.bincount(dst[src >= half_id], minlength=N)

    # per id-half, sort nodes by degree; deal blocks of 128 to the 4 cores of
    # that half round-robin.  node -> (core, block k, slot)
    node_core = np.zeros(N, np.int32)
    core_nodes = [[] for _ in range(NC)]
    for half in range(2):
        ids = np.arange(half_id) if half == 0 else np.arange(half_id, N)
        l_, h_ = lo_deg[ids], hi_deg[ids]
        key2 = np.where(l_ % 2 == 0, -h_, h_)
        order = ids[np.lexsort((key2, -l_))]
        padded = np.full(4 * CH_CAP, -1, np.int64)
        padded[:order.size] = order
        blocks = padded.reshape(-1, P)
        for g in range(blocks.shape[0]):
            c = half * 4 + (g % 4)
            blk = blocks[g]
            core_nodes[c].append(blk)
            node_core[blk[blk >= 0]] = c
    core_nodes = [np.concatenate(b) for b in core_nodes]   # [CH_CAP] ids (-1 pad)

    # table position of every node (rank-major: allgather layout)
    tab_pos = np.zeros(N, np.int64)
    for c in range(NC):
        blk = core_nodes[c]
        real = blk >= 0
        tab_pos[blk[real]] = c * CH_CAP + np.nonzero(real)[0]

    # the designated pad rows must not belong to real nodes
    assert core_nodes[NC // 2 - 1][-1] < 0, "lo pad row is a real node"
    assert core_nodes[NC - 1][-1] < 0, "hi pad row is a real node"

    # region widths per k (uniform over cores): max degree + 1 self slot
    Wlo = np.zeros(KB, np.int64)
    Whi = np.zeros(KB, np.int64)
    for c in range(NC):
        blk = core_nodes[c].reshape(KB, P)
        for k in range(KB):
            real = blk[k][blk[k] >= 0]
            if real.size:
                Wlo[k] = max(Wlo[k], lo_deg[real].max())
                Whi[k] = max(Whi[k], hi_deg[real].max())
    nlo = Wlo + 1
    nhi = Whi + 1

    # idx column base per (k, region) in the resident idx buffer
    colbase = []
    icol = 0
    for k in range(KB):
        colbase.append((icol, icol + 8 * int(nlo[k])))
        icol += 8 * int(nlo[k] + nhi[k])
    ICTOT = icol

    # CSR edge lists grouped by dst
    order_e = np.argsort(dst, kind="stable")
    src_s = src[order_e]
    dst_s = dst[order_e]
    seg_start = np.searchsorted(dst_s, np.arange(N))
    seg_end = np.searchsorted(dst_s, np.arange(N) + 1)

    def wrap(vals):
        # vals: flat [w*128] (slot-major); -> [128, 8*w] int16 gather layout
        NI = vals.size
        wrapped = vals.reshape(NI // 16, 16).T.astype(np.int16)
        return np.tile(wrapped, (8, 1))

    idx_bufs, xtt_bufs = [], []
    for core in range(NC):
        blk = core_nodes[core].reshape(KB, P)
        idx_buf = np.zeros((P, ICTOT), np.int16)
        for k in range(KB):
            wlo, whi = int(nlo[k]), int(nhi[k])
            lom = np.full((P, wlo), LO_PAD, np.int64)
            him = np.full((P, whi), HI_PAD, np.int64)
            for s in range(P):
                n = blk[k, s]
                if n < 0:
                    continue
                # self slot 0 of the region owning this core's nodes
                if core < NC // 2:
                    lom[s, 0] = tab_pos[n]
                else:
                    him[s, 0] = tab_pos[n] - HALFT
                es, ee = seg_start[n], seg_end[n]
                nbrs = tab_pos[src_s[es:ee]]
                nl = nbrs[nbrs < HALFT]
                nh = nbrs[nbrs >= HALFT] - HALFT
                lom[s, 1:1 + nl.size] = nl
                him[s, 1:1 + nh.size] = nh
            ic0, ich = colbase[k]
            idx_buf[:, ic0:ic0 + 8 * wlo] = wrap(lom.T.reshape(-1))
            idx_buf[:, ich:ich + 8 * whi] = wrap(him.T.reshape(-1))
        idx_bufs.append(idx_buf)

        # xT tiles [KB, 64, 128]
        KIN = 64
        xtt = np.zeros((KB, KIN, P), np.float32)
        for k in range(KB):
            for s in range(P):
                n = blk[k, s]
                if n >= 0:
                    xtt[k, :IN, s] = x[n]
        xtt_bufs.append(xtt)

    consts = {}
    w0e = _build_wext(params["w0"], params["att_src0"], params["att_dst0"])
    w0p = np.zeros((64, EXT), np.float32)
    w0p[:IN] = w0e
    consts["w0ext"] = w0p
    for l in (1, 2):
        we = _build_wext(params[f"w{l}"], params[f"att_src{l}"], params[f"att_dst{l}"])
        consts[f"w{l}ext"] = we.reshape(2, P, EXT).copy()
    consts["bias"] = np.stack([np.tile(params[f"b{l}"][None, :], (P, 1)) for l in range(3)])
    consts["identity"] = np.eye(P, dtype=np.float32)
    consts["hw1"] = np.asarray(params["head_w1"], np.float32).reshape(2, P, C)
    consts["hb1"] = np.tile(np.asarray(params["head_b1"], np.float32)[None, :], (P, 1))
    consts["hw2"] = np.asarray(params["head_w2"], np.float32).reshape(C, 1)
    hb2 = float(np.asarray(params["head_b2"]).reshape(-1)[0])

    # per-core a_src fixup for the last dense block: pad lanes get PAD_SCORE
    # added so every row they own reads as exp(score)=0 when gathered
    fixsrc_bufs = []
    for c in range(NC):
        blk = core_nodes[c].reshape(KB, P)
        fx = np.zeros((P, H), np.float32)
        fx[blk[KB - 1] < 0] = PAD_SCORE
        fixsrc_bufs.append(fx)

    geom = dict(N=N, KB=KB, CH_CAP=CH_CAP, TAB=TAB, HALFT=HALFT,
                ICTOT=ICTOT, hb2=hb2, colbase=colbase,
                nlo=tuple(int(v) for v in nlo), nhi=tuple(int(v) for v in nhi))
    return geom, idx_bufs, xtt_bufs, consts, core_nodes, fixsrc_bufs


def _build_program(geom):
    KB = geom["KB"]
    CH_CAP = geom["CH_CAP"]
    TAB = geom["TAB"]
    HALFT = geom["HALFT"]
    ICTOT = geom["ICTOT"]
    colbase = geom["colbase"]
    nlo, nhi = geom["nlo"], geom["nhi"]
    hb2 = geom["hb2"]
    WTMAX = max(a + b for a, b in zip(nlo, nhi))

    nc = bacc.Bacc("TRN2", target_bir_lowering=False, debug=False,
                   num_devices=NC, num_swdge_queues=4)

    xtt_d = nc.dram_tensor("xtt", [KB, 64, P], F32, kind="ExternalInput")
    idx_d = nc.dram_tensor("idxbuf", [P, ICTOT], I16, kind="ExternalInput")
    w0e_d = nc.dram_tensor("w0ext", [64, EXT], F32, kind="ExternalInput")
    w1e_d = nc.dram_tensor("w1ext", [2, P, EXT], F32, kind="ExternalInput")
    w2e_d = nc.dram_tensor("w2ext", [2, P, EXT], F32, kind="ExternalInput")
    bias_d = nc.dram_tensor("bias", [3, P, HC], F32, kind="ExternalInput")
    iden_d = nc.dram_tensor("identity", [P, P], F32, kind="ExternalInput")
    hw1_d = nc.dram_tensor("hw1", [2, P, C], F32, kind="ExternalInput")
    hb1_d = nc.dram_tensor("hb1", [P, C], F32, kind="ExternalInput")
    hw2_d = nc.dram_tensor("hw2", [C, 1], F32, kind="ExternalInput")
    fixsrc_d = nc.dram_tensor("fixsrc", [P, H], F32, kind="ExternalInput")
    out_d = nc.dram_tensor("outv", [CH_CAP, 1], F32, kind="ExternalOutput")

    qrr = [0]

    def next_q():
        q = qrr[0]
        qrr[0] = (q + 1) % 4
        return q

    with tile.TileContext(nc) as tc:
        with (
            tc.tile_pool(name="dram", bufs=1, space="DRAM") as dram,
            tc.tile_pool(name="consts", bufs=1) as cpool,
            tc.tile_pool(name="gp", bufs=4) as gp,
            tc.tile_pool(name="tp", bufs=2) as tp,
            tc.tile_pool(name="sp", bufs=4) as spool,
            tc.tile_pool(name="accp", bufs=3) as accp,
            tc.tile_pool(name="psum", bufs=2, space="PSUM") as pp,
            tc.tile_pool(name="psum2", bufs=2, space="PSUM") as pp2,
        ):
            bounce = [dram.tile([CH_CAP, ROW], BF, name=f"bounce{l}", tag=f"bounce{l}")
                      for l in range(3)]
            tabs = [dram.tile([TAB, ROW], BF, name=f"tab{l}", tag=f"tab{l}",
                              addr_space="Shared") for l in range(3)]

            w0e = cpool.tile([64, EXT], F32, name="w0e")
            nc.sync.dma_start(w0e[:], w0e_d[:])
            w1e = cpool.tile([P, 2, EXT], F32, name="w1e")
            nc.sync.dma_start(w1e[:], w1e_d[:].rearrange("a p e -> p a e"))
            w2e = cpool.tile([P, 2, EXT], F32, name="w2e")
            nc.sync.dma_start(w2e[:], w2e_d[:].rearrange("a p e -> p a e"))
            bias = cpool.tile([P, 3, HC], F32, name="bias")
            nc.sync.dma_start(bias[:], bias_d[:].rearrange("a p e -> p a e"))
            iden = cpool.tile([P, P], F32, name="iden")
            nc.sync.dma_start(iden[:], iden_d[:])
            hw1 = cpool.tile([P, 2, C], F32, name="hw1")
            nc.sync.dma_start(hw1[:], hw1_d[:].rearrange("a p e -> p a e"))
            hb1 = cpool.tile([P, C], F32, name="hb1")
            nc.sync.dma_start(hb1[:], hb1_d[:])
            hw2 = cpool.tile([C, 1], F32, name="hw2")
            nc.sync.dma_start(hw2[:], hw2_d[:])
            fixsrc = cpool.tile([P, H], F32, name="fixsrc")
            nc.sync.dma_start(fixsrc[:], fixsrc_d[:])
            idxall = cpool.tile([P, ICTOT], I16, name="idxall")
            nc.sync.dma_start(idxall[:], idx_d[:])
            sc_all = cpool.tile([P, KB, H], F32, name="sc_all")

            def emit_dense(ps, k, l):
                # pack psum [P, EXT] f32 -> table row + stash own a_dst
                if k == KB - 1:
                    # pad lanes' a_src -> -3e4 so their rows gather as alpha=0
                    nc.vector.tensor_tensor(out=ps[:, HC:HC + H], in0=ps[:, HC:HC + H],
                                            in1=fixsrc[:], op=AX.add)
                hb = spool.tile([P, ROW], BF, name="hb", tag="hb")
                if USE_SCALAR_COPIES:
                    nc.scalar.activation(hb[:, 0:NBF], ps[:, 0:NBF], ACT.Copy)
                    nc.vector.tensor_copy(out=hb[:, NBF:NBF + 8].bitcast(FP8),
                                          in_=ps[:, NBF:HC])
                    nc.scalar.activation(hb[:, NBF + 8:ROW].bitcast(F32),
                                         ps[:, HC:HC + H], ACT.Copy)
                    nc.scalar.activation(sc_all[:, k, :], ps[:, HC + H:EXT], ACT.Copy)
                else:
                    nc.vector.tensor_copy(out=hb[:, 0:NBF], in_=ps[:, 0:NBF])
                    nc.vector.tensor_copy(out=hb[:, NBF:NBF + 8].bitcast(FP8),
                                          in_=ps[:, NBF:HC])
                    nc.vector.tensor_copy(out=hb[:, NBF + 8:ROW].bitcast(F32),
                                          in_=ps[:, HC:HC + H])
                    nc.vector.tensor_copy(out=sc_all[:, k, :], in_=ps[:, HC + H:EXT])
                nc.sync.dma_start(bounce[l][k * P:(k + 1) * P, :], hb[:])

            def allgather(l):
                nc.gpsimd.collective_compute(
                    "AllGather", AX.bypass, replica_groups=[list(range(NC))],
                    ins=[bounce[l].opt()], outs=[tabs[l].opt()])

            # ---- layer-0 dense phase: h0 = x @ W0ext (own nodes only)
            for k in range(KB):
                xt = spool.tile([64, P], F32, name="xt", tag="xt")
                nc.sync.dma_start(xt[:], xtt_d[k])
                ps = pp.tile([P, EXT], F32, name="psmm", tag="psmm")
                nc.tensor.matmul(ps[:], lhsT=xt[:], rhs=w0e[:], start=True, stop=True)
                emit_dense(ps, k, 0)
            allgather(0)

            # ---- 3 GAT layers
            for l in range(3):
                tab = tabs[l]
                for k in range(KB):
                    wlo, whi = nlo[k], nhi[k]
                    wt = wlo + whi
                    ic0, ich = colbase[k]
                    g = gp.tile([P, WTMAX, ROW], BF, name="g", tag="g")
                    for (src0, src1, w_r, icb, gcol) in (
                            (0, HALFT, wlo, ic0, 0), (HALFT, TAB, whi, ich, wlo)):
                        col = 0
                        while col < w_r:
                            w = min(GCAP, w_r - col)
                            nc.gpsimd.dma_gather(
                                out_ap=g[:, gcol + col:gcol + col + w, :],
                                in_ap=tab[src0:src1, :],
                                idxs_ap=idxall[:, icb + 8 * col:icb + 8 * (col + w)],
                                num_idxs=P * w, num_idxs_reg=P * w, elem_size=ROW,
                                queue_num=next_q())
                            col += w

                    # per-edge scores: a_src (gathered) + a_dst (own)
                    sw = spool.tile([P, WTMAX, H], F32, name="sw", tag="sw")
                    nc.vector.tensor_tensor(
                        out=sw[:, 0:wt, :],
                        in0=g[:, 0:wt, NBF + 8:ROW].bitcast(F32),
                        in1=sc_all[:, k, :].unsqueeze(1).to_broadcast([P, wt, H]),
                        op=AX.add)
                    if USE_SCALAR_LRELU:
                        nc.scalar.activation(sw[:, 0:wt, :], sw[:, 0:wt, :],
                                             ACT.Lrelu, alpha=NEG_SLOPE)
                    else:
                        sb2 = spool.tile([P, WTMAX, H], F32, name="sb2", tag="sb2")
                        nc.vector.tensor_scalar(out=sb2[:, 0:wt, :], in0=sw[:, 0:wt, :],
                                                scalar1=NEG_SLOPE, scalar2=None,
                                                op0=AX.mult)
                        nc.vector.tensor_tensor(out=sw[:, 0:wt, :], in0=sw[:, 0:wt, :],
                                                in1=sb2[:, 0:wt, :], op=AX.max)
                    swb = spool.tile([P, WTMAX, H], BF, name="swb", tag="swb")
                    dn = spool.tile([P, H], F32, name="dn", tag="dn")
                    if USE_ACCUM:
                        for h in range(H):
                            nc.scalar.activation(swb[:, 0:wt, h], sw[:, 0:wt, h],
                                                 ACT.Exp, accum_out=dn[:, h:h + 1])
                    else:
                        nc.scalar.activation(swb[:, 0:wt, :], sw[:, 0:wt, :], ACT.Exp)
                        nc.vector.tensor_reduce(
                            out=dn[:], in_=swb[:, 0:wt, :].rearrange("p w h -> p h w"),
                            axis=mybir.AxisListType.X, op=AX.add)
                    # clamp so all-pad lanes give 0/tiny = 0 instead of NaN
                    nc.vector.tensor_scalar(out=dn[:], in0=dn[:], scalar1=1e-30,
                                            scalar2=None, op0=AX.max)
                    rec = spool.tile([P, H], F32, name="rec", tag="rec")
                    nc.vector.reciprocal(rec[:], dn[:])

                    # weighted messages tmp = h_src * exp(score)
                    tmp = tp.tile([P, WTMAX, HC], BF, name="tmp", tag="tmp")
                    nc.vector.tensor_tensor(
                        out=tmp[:, 0:wt, 0:192].rearrange("p w (h c) -> p w h c", h=3),
                        in0=g[:, 0:wt, 0:192].rearrange("p w (h c) -> p w h c", h=3),
                        in1=swb[:, 0:wt, 0:3].unsqueeze(3).to_broadcast([P, wt, 3, C]),
                        op=AX.mult)
                    nc.vector.tensor_tensor(
                        out=tmp[:, 0:wt, 192:NBF],
                        in0=g[:, 0:wt, 192:NBF],
                        in1=swb[:, 0:wt, 3].unsqueeze(2).to_broadcast([P, wt, NBF - 192]),
                        op=AX.mult)
                    g8 = spool.tile([P, WTMAX, NF8], BF, name="g8", tag="g8")
                    nc.vector.tensor_copy(out=g8[:, 0:wt, :],
                                          in_=g[:, 0:wt, NBF:NBF + 8].bitcast(FP8))
                    nc.vector.tensor_tensor(
                        out=tmp[:, 0:wt, NBF:HC],
                        in0=g8[:, 0:wt, :],
                        in1=swb[:, 0:wt, 3].unsqueeze(2).to_broadcast([P, wt, NF8]),
                        op=AX.mult)

                    num = accp.tile([P, HC], F32, name="num", tag="num")
                    nc.vector.tensor_reduce(
                        out=num[:], in_=tmp[:, 0:wt, :].rearrange("p w x -> p x w"),
                        axis=mybir.AxisListType.X, op=AX.add)

                    # epilogue: y = ELU(num/dn + bias)
                    y = spool.tile([P, HC], F32, name="y", tag="y")
                    nc.vector.tensor_tensor(
                        out=y[:].rearrange("p (h c) -> p h c", h=H),
                        in0=num[:].rearrange("p (h c) -> p h c", h=H),
                        in1=rec[:].unsqueeze(2).to_broadcast([P, H, C]),
                        op=AX.mult)
                    nc.vector.tensor_tensor(out=y[:], in0=y[:], in1=bias[:, l, :], op=AX.add)
                    yneg = spool.tile([P, HC], F32, name="yneg", tag="yneg")
                    nc.vector.tensor_scalar(out=yneg[:], in0=y[:], scalar1=0.0,
                                            scalar2=None, op0=AX.min)
                    nc.scalar.activation(yneg[:], yneg[:], ACT.Exp)
                    nc.vector.tensor_scalar(out=y[:], in0=y[:], scalar1=0.0,
                                            scalar2=-1.0, op0=AX.max, op1=AX.add)
                    nc.vector.tensor_tensor(out=y[:], in0=y[:], in1=yneg[:], op=AX.add)

                    # transpose y for the next dense matmul
                    yt = spool.tile([P, HC], F32, name="yt", tag="yt")
                    for half in range(2):
                        pt = pp2.tile([P, P], F32, name="pt", tag="pt")
                        nc.tensor.transpose(out=pt[:], in_=y[:, half * P:(half + 1) * P],
                                            identity=iden[:])
                        if USE_SCALAR_COPIES:
                            nc.scalar.activation(yt[:, half * P:(half + 1) * P], pt[:],
                                                 ACT.Copy)
                        else:
                            nc.vector.tensor_copy(out=yt[:, half * P:(half + 1) * P],
                                                  in_=pt[:])
                    if l < 2:
                        we = w1e if l == 0 else w2e
                        ps = pp.tile([P, EXT], F32, name="psmm", tag="psmm")
                        nc.tensor.matmul(ps[:], lhsT=yt[:, 0:P], rhs=we[:, 0, :],
                                         start=True, stop=False)
                        nc.tensor.matmul(ps[:], lhsT=yt[:, P:HC], rhs=we[:, 1, :],
                                         start=False, stop=True)
                        emit_dense(ps, k, l + 1)
                    else:
                        zp = pp2.tile([P, C], F32, name="zp", tag="pt")
                        nc.tensor.matmul(zp[:], lhsT=yt[:, 0:P], rhs=hw1[:, 0, :],
                                         start=True, stop=False)
                        nc.tensor.matmul(zp[:], lhsT=yt[:, P:HC], rhs=hw1[:, 1, :],
                                         start=False, stop=True)
                        z = spool.tile([P, C], F32, name="z", tag="z")
                        nc.vector.tensor_tensor(out=z[:], in0=zp[:], in1=hb1[:], op=AX.add)
                        nc.scalar.activation(z[:], z[:], ACT.Relu)
                        ztp = pp2.tile([P, P], F32, name="ztp", tag="pt")
                        nc.tensor.transpose(out=ztp[0:C, 0:P], in_=z[:, 0:C], identity=iden[:])
                        zt = spool.tile([C, P], F32, name="zt", tag="zt")
                        if USE_SCALAR_COPIES:
                            nc.scalar.activation(zt[:], ztp[0:C, 0:P], ACT.Copy)
                        else:
                            nc.vector.tensor_copy(out=zt[:], in_=ztp[0:C, 0:P])
                        op_ = pp2.tile([P, 1], F32, name="op_", tag="pt")
                        nc.tensor.matmul(op_[:], lhsT=zt[:], rhs=hw2[:], start=True, stop=True)
                        o = spool.tile([P, 1], F32, name="o", tag="o")
                        nc.vector.tensor_scalar(out=o[:], in0=op_[:], scalar1=hb2,
                                                scalar2=None, op0=AX.add)
                        nc.sync.dma_start(out_d[k * P:(k + 1) * P, :], o[:])
                if l < 2:
                    allgather(l + 1)

    nc.compile()
    return nc


def kernel(**inputs):
    x = np.asarray(inputs["x"], np.float32)
    edge_index = np.asarray(inputs["edge_index"])
    params = {k: np.asarray(v) for k, v in inputs.items() if k not in ("x", "edge_index")}

    geom, idx_bufs, xtt_bufs, consts, core_nodes, fixsrc_bufs = _host_prep(
        x, edge_index, params)

    key = (geom["N"], geom["KB"], geom["nlo"], geom["nhi"])
    if key not in _cache:
        _cache[key] = _build_program(geom)
    nc = _cache[key]

    in_maps = []
    for c in range(NC):
        in_maps.append({
            "xtt": xtt_bufs[c],
            "idxbuf": idx_bufs[c],
            "w0ext": consts["w0ext"],
            "w1ext": consts["w1ext"],
            "w2ext": consts["w2ext"],
            "bias": consts["bias"],
            "identity": consts["identity"],
            "hw1": consts["hw1"],
            "hb1": consts["hb1"],
            "hw2": consts["hw2"],
            "fixsrc": fixsrc_bufs[c],
        })
    import os
    trace = os.environ.get("GAT_KERNEL_TRACE") == "1"
    res = bass_utils.run_bass_kernel_spmd(nc, in_maps, core_ids=list(range(NC)),
                                          trace=trace)
    kernel._last_exec_ns = res.exec_time_ns
    out = np.zeros(geom["N"], np.float32)
    for c in range(NC):
        blk = core_nodes[c]
        real = blk >= 0
        out[blk[real]] = res.results[c]["outv"][:, 0][real]
    return out
